# revision 1
# baseline (speedup 1.0000x reference)
"""Trainium2 Bass kernel for a 2-layer bidirectional LSTM encoder.

Problem: x [256, 2048, 64] -> bilstm(H=4) -> [.,.,8] -> bilstm(H=2) -> [256, 2048, 4]

Strategy (8 cores, pure data parallel over batch, 32 seqs/core):
- All sigmoids become tanh via sigmoid(z) = (tanh(z/2)+1)/2 so the scalar
  engine runs a single activation table the whole kernel.
- Carried state is h~ = 2h and c~ = 2c; the 0.5 factors fold into Whh
  (0.5*Whh), the per-row ACT scale (0.5 on i,f,o rows), and one final
  tensor_scalar 0.5 multiply for the layer-2 output.
- Gate rows live at partition quads (i@0, f@32, o@64, g@96) of the PSUM
  matmul output / tanh tile; GPSIMD cross-partition copies bring f,o,g
  down to base partition 0 where all the vector-engine cell ops run
  (HW requires all SBUF operands of an op to share a start partition,
  and starts must be one of {0,32,64,96}).
- fwd and bwd directions are two independent chains that pipeline
  across PE/ACT/DVE/Pool.
- Input projections are computed 16 steps at a time as one PE GEMM into
  a PSUM bank, from x^T windows built with DVE stream (32x32 block)
  transposes.  Layer-1 hidden states stay resident in SBUF, stacked
  time-quarter-per-partition-quad, and are staged back to base 0 for
  layer-2's input GEMMs.
"""

import numpy as np

_B, _S, _F = 256, 2048, 64
_H1, _H2 = 4, 2
_NC = 8
_BL = _B // _NC  # 32


# ---------------------------------------------------------------- host-side
def _quad_pack(Wih, Whh, bih, bhh, H, Fin, wih_scale=1.0):
    """Pack one direction's weights into the partition-quad row layout.

    Returns (WihQ [M,Fin], WhhQ [M,H], scale [M,1], bias [M,1]) with
    M = 96+H; gate rows at quads i@0, f@32, o@64, g@96 (PyTorch source
    order is i,f,g,o).  WhhQ absorbs the h~=2h halving; ACT row scale
    applies the tanh-trick halving on i,f,o rows.
    """
    M = 96 + H
    WihQ = np.zeros((M, Fin), np.float32)
    WhhQ = np.zeros((M, H), np.float32)
    scale = np.zeros((M, 1), np.float32)
    bias = np.zeros((M, 1), np.float32)
    bsum = (np.asarray(bih) + np.asarray(bhh)).astype(np.float32)
    for quad, blk, sc in ((0, 0, 0.5), (32, 1, 0.5), (64, 3, 0.5), (96, 2, 1.0)):
        WihQ[quad:quad + H] = wih_scale * np.asarray(Wih)[blk * H:(blk + 1) * H]
        WhhQ[quad:quad + H] = 0.5 * np.asarray(Whh)[blk * H:(blk + 1) * H]
        scale[quad:quad + H, 0] = sc
        bias[quad:quad + H, 0] = sc * bsum[blk * H:(blk + 1) * H]
    return WihQ, WhhQ, scale, bias


def _pack_weights(inp):
    out = {}
    for d, sfx in (("f", "_f"), ("b", "_b")):
        WihQ, WhhQ, sc, bi = _quad_pack(
            inp["l1_Wih" + sfx], inp["l1_Whh" + sfx],
            inp["l1_bih" + sfx], inp["l1_bhh" + sfx], _H1, _F)
        out[f"l1{d}_wih"] = np.ascontiguousarray(WihQ.T)      # [64, M1]
        out[f"l1{d}_whh"] = np.ascontiguousarray(WhhQ.T)      # [4, M1]
        out[f"l1{d}_scale"] = sc
        out[f"l1{d}_bias"] = bi
        WihQ2, WhhQ2, sc2, bi2 = _quad_pack(
            inp["l2_Wih" + sfx], inp["l2_Whh" + sfx],
            inp["l2_bih" + sfx], inp["l2_bhh" + sfx], _H2, 2 * _H1,
            wih_scale=0.5)  # layer-2 input is h~1 = 2*h1
        out[f"l2{d}_wih_hf"] = np.ascontiguousarray(WihQ2[:, 0:_H1].T)  # [4, M2]
        out[f"l2{d}_wih_hb"] = np.ascontiguousarray(WihQ2[:, _H1:2 * _H1].T)
        out[f"l2{d}_whh"] = np.ascontiguousarray(WhhQ2.T)     # [2, M2]
        out[f"l2{d}_scale"] = sc2
        out[f"l2{d}_bias"] = bi2
    return out


def _wspec():
    M1, M2 = 96 + _H1, 96 + _H2
    spec = {}
    for d in ("f", "b"):
        spec[f"l1{d}_wih"] = [_F, M1]
        spec[f"l1{d}_whh"] = [_H1, M1]
        spec[f"l1{d}_scale"] = [M1, 1]
        spec[f"l1{d}_bias"] = [M1, 1]
        spec[f"l2{d}_wih_hf"] = [_H1, M2]
        spec[f"l2{d}_wih_hb"] = [_H1, M2]
        spec[f"l2{d}_whh"] = [_H2, M2]
        spec[f"l2{d}_scale"] = [M2, 1]
        spec[f"l2{d}_bias"] = [M2, 1]
    return spec


# ---------------------------------------------------------------- device
def _build(S):
    import concourse.bacc as bacc
    import concourse.mybir as mybir
    from concourse.tile import TileContext
    from contextlib import ExitStack

    fp32 = mybir.dt.float32
    Tanh = mybir.ActivationFunctionType.Tanh
    Alu = mybir.AluOpType
    BL, F, H1, H2 = _BL, _F, _H1, _H2
    M1, M2 = 96 + H1, 96 + H2
    W = 16                # timesteps per PSUM window
    NW = S // W
    Q = S // 4            # timesteps per H1 partition-quarter
    assert S % 64 == 0

    nc = bacc.Bacc(None, target_bir_lowering=False)
    x = nc.dram_tensor("x", [BL, S, F], fp32, kind="ExternalInput")
    outd = {"f": nc.dram_tensor("outf", [2, S, BL], fp32, kind="ExternalOutput"),
            "b": nc.dram_tensor("outb", [2, S, BL], fp32, kind="ExternalOutput")}
    wdram = {k: nc.dram_tensor(k, shp, fp32, kind="ExternalInput")
             for k, shp in _wspec().items()}

    with TileContext(nc) as tc, ExitStack() as ctx:
        wpool = ctx.enter_context(tc.tile_pool(name="wpool", bufs=1))
        h1pool = ctx.enter_context(tc.tile_pool(name="h1pool", bufs=1))
        spool = ctx.enter_context(tc.tile_pool(name="spool", bufs=1))
        xpool = ctx.enter_context(tc.tile_pool(name="xpool", bufs=4))
        xtpool = ctx.enter_context(tc.tile_pool(name="xtpool", bufs=4))
        opool = ctx.enter_context(tc.tile_pool(name="opool", bufs=4))
        hstpool = ctx.enter_context(tc.tile_pool(name="hstpool", bufs=8))
        ppool = ctx.enter_context(tc.tile_pool(name="ppool", bufs=4, space="PSUM"))

        wtile = {}
        for k, shp in _wspec().items():
            t = wpool.tile(shp, fp32, name=k)
            nc.sync.dma_start(t[:], wdram[k][:])
            wtile[k] = t

        H1sb = {"f": h1pool.tile([128, Q * BL], fp32, name="H1f"),
                "b": h1pool.tile([128, Q * BL], fp32, name="H1b")}

        def h1_slice(d, t, n=1):
            q = 32 * (t // Q)
            c0 = (t % Q) * BL
            return H1sb[d][q:q + H1, c0:c0 + n * BL]

        def make_state(H, tag):
            st = {}
            for d in ("f", "b"):
                st[d] = {k: spool.tile([H, BL], fp32, name=f"{k}{tag}{d}")
                         for k in ("G0", "F0", "O0", "T1", "T2", "C", "TC", "HS")}
                st[d]["TG"] = spool.tile([128, BL], fp32, name=f"TG{tag}{d}")
            return st

        def cell(st, p, pos, M, H, wpfx, s, j):
            """One LSTM cell step: gates tanh + cell update.  Returns after
            st['HS'] holds h~_t (at base partition 0)."""
            TG = st["TG"]
            nc.scalar.activation(TG[0:M, :], p[0:M, pos:pos + BL], Tanh,
                                 bias=wtile[wpfx + "_bias"][:, :],
                                 scale=wtile[wpfx + "_scale"][:, :])
            # bring f', o', g down to base partition 0 (Pool cross-quad copies)
            nc.gpsimd.tensor_copy(st["G0"][:, :], TG[96:96 + H, :])
            nc.gpsimd.tensor_copy(st["F0"][:, :], TG[32:32 + H, :])
            nc.gpsimd.tensor_copy(st["O0"][:, :], TG[64:64 + H, :])
            if s == 0:
                nc.vector.scalar_tensor_tensor(
                    st["C"][:, :], TG[0:H, :], 1.0, st["G0"][:, :],
                    Alu.add, Alu.mult)
            else:
                nc.vector.scalar_tensor_tensor(
                    st["T1"][:, :], TG[0:H, :], 1.0, st["G0"][:, :],
                    Alu.add, Alu.mult)
                nc.vector.scalar_tensor_tensor(
                    st["T2"][:, :], st["F0"][:, :], 1.0, st["C"][:, :],
                    Alu.add, Alu.mult)
                nc.vector.scalar_tensor_tensor(
                    st["C"][:, :], st["T2"][:, :], 0.5, st["T1"][:, :],
                    Alu.mult, Alu.add)
            nc.scalar.activation(st["TC"][:, :], st["C"][:, :], Tanh, scale=0.5)
            nc.vector.scalar_tensor_tensor(
                st["HS"][:, :], st["O0"][:, :], 1.0, st["TC"][:, :],
                Alu.add, Alu.mult)

        # ------------------------------------------------ phase 1: layer 1
        st1 = make_state(H1, "1")
        for w in range(NW):
            pw = {}
            for d in ("f", "b"):
                t_lo = w * W if d == "f" else S - (w + 1) * W
                xl = xpool.tile([64, W * F], fp32, name=f"xl{d}", tag="xl")
                xlv = xl[:].rearrange("p (t f) -> p t f", t=W)
                nc.sync.dma_start(xlv[0:32], x[:, t_lo:t_lo + W, :])
                nc.sync.dma_start(xlv[32:64], x[:, t_lo:t_lo + W, :])
                xT = xtpool.tile([F, W * BL], fp32, name=f"xT{d}", tag="xT")
                nc.vector.transpose(xT[0:32, :], xlv[0:32, :, 0:32])
                nc.vector.transpose(xT[32:64, :], xlv[32:64, :, 32:64])
                p = ppool.tile([128, W * BL], fp32, name=f"p1{d}", tag="pp")
                nc.tensor.matmul(p[0:M1, :], wtile[f"l1{d}_wih"][:, :], xT[:, :],
                                 start=True, stop=False)
                pw[d] = p
            for j in range(W):
                for d in ("f", "b"):
                    s = w * W + j
                    t = s if d == "f" else S - 1 - s
                    t_lo = w * W if d == "f" else S - (w + 1) * W
                    pos = (t - t_lo) * BL
                    st, p = st1[d], pw[d]
                    if s > 0:
                        nc.tensor.matmul(p[0:M1, pos:pos + BL],
                                         wtile[f"l1{d}_whh"][:, :],
                                         st["HS"][:, :],
                                         start=False, stop=(j == W - 1))
                    cell(st, p, pos, M1, H1, f"l1{d}", s, j)
                    # h~1 -> resident quad-stacked H1 buffer (for layer 2)
                    nc.gpsimd.tensor_copy(h1_slice(d, t), st["HS"][:, :])

        # ------------------------------------------------ phase 2: layer 2
        st2 = make_state(H2, "2")
        for w in range(NW):
            pw = {}
            ost = {}
            for d in ("f", "b"):
                t_lo = w * W if d == "f" else S - (w + 1) * W
                # stage quad-stacked H1 windows down to base partition 0 so
                # they are legal matmul rhs operands
                hstf = hstpool.tile([H1, W * BL], fp32, name=f"hstf{d}", tag="hst")
                hstb = hstpool.tile([H1, W * BL], fp32, name=f"hstb{d}", tag="hst")
                nc.gpsimd.tensor_copy(hstf[:, :], h1_slice("f", t_lo, n=W))
                nc.gpsimd.tensor_copy(hstb[:, :], h1_slice("b", t_lo, n=W))
                p = ppool.tile([128, W * BL], fp32, name=f"p2{d}", tag="pp")
                nc.tensor.matmul(p[0:M2, :], wtile[f"l2{d}_wih_hf"][:, :],
                                 hstf[:, :], start=True, stop=False)
                nc.tensor.matmul(p[0:M2, :], wtile[f"l2{d}_wih_hb"][:, :],
                                 hstb[:, :], start=False, stop=False)
                pw[d] = p
                ost[d] = opool.tile([2, W * BL], fp32, name=f"os{d}", tag="os")
            for j in range(W):
                for d in ("f", "b"):
                    s = w * W + j
                    t = s if d == "f" else S - 1 - s
                    t_lo = w * W if d == "f" else S - (w + 1) * W
                    pos = (t - t_lo) * BL
                    st, p = st2[d], pw[d]
                    if s > 0:
                        nc.tensor.matmul(p[0:M2, pos:pos + BL],
                                         wtile[f"l2{d}_whh"][:, :],
                                         st["HS"][:, :],
                                         start=False, stop=(j == W - 1))
                    cell(st, p, pos, M2, H2, f"l2{d}", s, j)
                    # true h2 = 0.5 * h~2 into the output staging tile
                    nc.vector.tensor_scalar_mul(ost[d][:, pos:pos + BL],
                                                st["HS"][:, :], 0.5)
            for d in ("f", "b"):
                t_lo = w * W if d == "f" else S - (w + 1) * W
                nc.sync.dma_start(outd[d][:, t_lo:t_lo + W, :],
                                  ost[d][:].rearrange("p (t b) -> p t b", t=W))
    nc.finalize()
    return nc


# ---------------------------------------------------------------- entry
def _run(x_full, packed, S, n_cores, _return_res=False, **runkw):
    from concourse.bass_utils import run_bass_kernel_spmd
    nc = _build(S)
    in_maps = []
    for c in range(n_cores):
        m = {k: v for k, v in packed.items()}
        m["x"] = np.ascontiguousarray(
            x_full[c * _BL:(c + 1) * _BL, :S], np.float32)
        in_maps.append(m)
    res = run_bass_kernel_spmd(nc, in_maps, core_ids=list(range(n_cores)), **runkw)
    nb = n_cores * _BL
    out = np.zeros((nb, S, 4), np.float32)
    for c in range(n_cores):
        r = res.results[c]
        out[c * _BL:(c + 1) * _BL, :, 0:2] = r["outf"].transpose(2, 1, 0)
        out[c * _BL:(c + 1) * _BL, :, 2:4] = r["outb"].transpose(2, 1, 0)
    if _return_res:
        return out, res
    return out


def kernel(**inputs):
    packed = _pack_weights(inputs)
    x = np.asarray(inputs["x"], np.float32)
    return _run(x, packed, _S, _NC)

